# revision 23
# baseline (speedup 1.0000x reference)
"""Trainium2 Bass kernel for nn_BERTCharting (pairwise-concat MLP).

Reference computation (per batch b):
    p = repr_w[b] @ W1[:H]        # [N, HID]
    q = repr_w[b] @ W1[H:]        # [N, HID]
    h[i,j,:] = relu(p[j] + q[i] + b1)
    out[i,j,:] = h[i,j] @ W2 + b2

Sharding: data-parallel over batch B=8 across the 8 NeuronCores (one batch
element per core). No collectives.

Design (per core; steady state measured DVE 100% / ACT ~100% busy):
  - inputs host-packed so every DMA descriptor is a contiguous >=512B
    per-partition row at full HWDGE rate; load order W1[d=0] -> reprT
    -> W1[d=1,2] (sync queue) so the first GEMM's d=0 chain (and then
    h-gen) starts ~4us after the fixed ~6.7us NEFF preamble; W2
    prefetches on the gpsimd (SWDGE) queue.
  - first GEMM on PE -> pT[d] bf16 / qbT[d]=qT+b1 fp32 (d-sequential;
    p/q accumulation chains interleaved so the PE pipeline overlaps).
  - h-gen: per-i ops [128 dpart, 128 j]: the per-partition scalar
    q[d,i] caps the free dim at N=128 (measured: FD=512 tensor_scalar
    hits the DVE 4x perf mode, but per-i scalars make it unusable, so
    the stream runs at the per-op floor). DVE dual-op tensor_scalar
    ~163 ns/op pipelined; ACT activation(Relu,bias) ~292 ns/op takes
    one il-slice of every h4 tile (12/48 ops) so no single tile is a
    long pole for the in-order PE queue. (GPSIMD tensor_scalar was
    measured at ~2.1us/op and contends with DVE's SBUF port - unusable.)
  - main GEMM: 8 mega-tiles of 16 i's, PSUM po[100, 2048] (4 banks,
    double-buffered), d-OUTER matmul order (stationary W2[d] held for 4
    MMs into different 512-col slices -> no PSUM output dep between
    neighbours). Last tile evicts in four 512-col slices to shorten
    the pipeline drain.
  - eviction: ACT copy [100, 2048] PSUM->SBUF bf16 (~1.9us/tile; DMA
    and GPSIMD have no PSUM route, so eviction must burn ACT/DVE
    cycles), then one DMA with 100 contiguous 4KiB descriptors into
    outT[l, i, j] bf16 (host upcasts + transposes to [i,j,l]; bf16
    halves output DMA, rel err 0.0021 -> 0.0033, budget 2e-2).
  - Note: the chip DVFS-throttles sustained activity (50%-duty windows
    of 11-28us per run), so exec_time is noisy +-2us run-to-run.
"""

import os
import sys

for _p in ("/opt/trn_rl_repo",):
    if _p not in sys.path and os.path.isdir(_p):
        sys.path.insert(0, _p)

import numpy as np
import ml_dtypes

import concourse.mybir as mybir
from concourse import bacc, bass
from concourse.tile import TileContext
from concourse.bass_utils import run_bass_kernel_spmd


def _ensure_ntff_hook():
    """Provide antenv.axon_hooks (NTFF profile get/set) if the image lacks it,
    and install the ctypes-based profile hook against libaxon_pjrt.so so that
    run_bass_kernel_spmd(trace=True) can capture hardware profiles."""
    try:
        from antenv.axon_hooks import get_axon_ntff_profile_hook  # noqa: F401
        return
    except ImportError:
        pass
    import contextlib
    import ctypes
    import types

    mod = types.ModuleType("antenv.axon_hooks")
    holder = {"hook": None}
    mod.set_axon_ntff_profile_hook = lambda h: holder.__setitem__("hook", h)
    mod.get_axon_ntff_profile_hook = lambda: holder["hook"]
    sys.modules["antenv.axon_hooks"] = mod
    try:
        import antenv
        antenv.axon_hooks = mod
    except ImportError:
        pass

    so_path = "/opt/axon/libaxon_pjrt.so"
    if not os.path.exists(so_path):
        return
    lib = ctypes.CDLL(so_path)
    if not hasattr(lib, "axon_start_nrt_profile"):
        return
    lib.axon_start_nrt_profile.argtypes = [
        ctypes.POINTER(ctypes.c_int64),
        ctypes.c_size_t,
    ]
    lib.axon_start_nrt_profile.restype = ctypes.c_int64
    lib.axon_stop_nrt_profile.argtypes = [ctypes.c_char_p]
    lib.axon_stop_nrt_profile.restype = ctypes.c_int64

    @contextlib.contextmanager
    def _hook(output_dir, device_ids):
        import jax

        jax.devices()
        if device_ids:
            ids = (ctypes.c_int64 * len(device_ids))(*device_ids)
            rc = lib.axon_start_nrt_profile(ids, len(device_ids))
        else:
            rc = lib.axon_start_nrt_profile(None, 0)
        if rc != 0:
            raise RuntimeError(f"axon_start_nrt_profile rc={rc}")
        try:
            yield
        finally:
            n = lib.axon_stop_nrt_profile(str(output_dir).encode())
            print(f"ntff profile: {n} file(s) written to {output_dir}",
                  file=sys.stderr)

    mod.set_axon_ntff_profile_hook(_hook)


_ensure_ntff_hook()

B, N, H = 8, 128, 768
HID, L = 384, 100
NCORES = 8
KT = H // 128          # 6 contraction tiles for the first GEMM
DT = HID // 128        # 3 d-tiles
GROUP = 4              # i's per 512-col psum slice
TILE_G = 4             # groups per psum mega-tile (4 banks)
TILE_I = GROUP * TILE_G        # 16 i's per mega-tile
NTILES = N // TILE_I           # 8 mega-tiles

F32 = mybir.dt.float32
BF16 = mybir.dt.bfloat16

# Of the 48 h-gen ops per mega-tile, this many go to ACT (rest DVE);
# balances DVE (~163ns/op) vs ACT (~292ns/op + ~16us eviction load).
ACT_OPS_PER_TILE = 12

# Stash of the last run's BassKernelResults (test harness reads exec_time_ns).
LAST_RESULT = None


def _build_program():
    nc = bacc.Bacc(None, target_bir_lowering=False)

    # Host-packed layouts: per-partition rows contiguous in DRAM.
    reprP = nc.declare_dram_parameter("reprP", [128, KT * N], BF16,
                                      isOutput=False)
    # w1p[d][p, (half,k)*128+c] = W1[half*H + k*128 + p, d*128 + c]
    w1p = nc.declare_dram_parameter("w1p", [DT, 128, 2 * KT * 128], BF16,
                                    isOutput=False)
    # w2p[p, d*L+l] = W2[d*128+p, l]
    w2p = nc.declare_dram_parameter("w2p", [128, DT * L], BF16,
                                    isOutput=False)
    b1c = nc.declare_dram_parameter("b1c", [128, DT], F32, isOutput=False)
    # Output l-major bf16: outT[l, i, j]; host upcasts + transposes to
    # [i, j, l]. bf16 halves the output DMA (~6.5MB -> 3.3MB per core);
    # the added rounding is ~0.4% of scale, well under the 2e-2 budget.
    outT = nc.declare_dram_parameter("outT", [L, N, N], BF16, isOutput=True)

    add = mybir.AluOpType.add
    maxop = mybir.AluOpType.max

    with TileContext(nc) as tc:
        with tc.tile_pool(name="const", bufs=1) as cpool:
            # ---- input loads: one full-rate DMA per chunk ------------------
            # w1[d=0] first so the first GEMM's d=0 chain starts earliest;
            # b1/w2 issued from the idle gpsimd queue to unclog sync.
            w1_sb = []
            w1_tiles = [
                cpool.tile([128, 2 * KT, 128], BF16, tag=f"w1d{d}",
                           name=f"w1d{d}")
                for d in range(DT)
            ]
            nc.sync.dma_start(
                out=w1_tiles[0],
                in_=w1p[0, :, :].rearrange("p (q c) -> p q c", q=2 * KT),
            )
            reprT_big = cpool.tile([128, KT, N], BF16, tag="reprTb",
                                   name="reprTb")
            nc.sync.dma_start(
                out=reprT_big,
                in_=reprP[:].rearrange("p (k n) -> p k n", k=KT),
            )
            reprT_sb = [reprT_big[:, k, :] for k in range(KT)]
            for d in range(1, DT):
                nc.sync.dma_start(
                    out=w1_tiles[d],
                    in_=w1p[d, :, :].rearrange("p (q c) -> p q c", q=2 * KT),
                )
            w1_sb = w1_tiles
            b1_sb = cpool.tile([128, DT], F32, tag="b1c", name="b1sb")
            nc.gpsimd.dma_start(out=b1_sb, in_=b1c[:, :])
            w2_big = cpool.tile([128, DT, L], BF16, tag="w2b", name="w2b")
            nc.gpsimd.dma_start(
                out=w2_big,
                in_=w2p[:].rearrange("p (d l) -> p d l", d=DT),
            )
            w2_sb = [w2_big[:, d, :] for d in range(DT)]

            # ---- first GEMMs: pT, qbT (d-sequential, p/q interleaved) -----
            pT, qbT = [], []
            with tc.tile_pool(name="ps1", bufs=1, space="PSUM") as ps1:
                for d in range(DT):
                    pp = ps1.tile([128, N], F32, tag="pp", name=f"pp{d}",
                                  bufs=2)
                    pq = ps1.tile([128, N], F32, tag="pq", name=f"pq{d}",
                                  bufs=2)
                    for k in range(KT):
                        nc.tensor.matmul(
                            pp,
                            lhsT=w1_sb[d][:, k, :],
                            rhs=reprT_sb[k],
                            start=(k == 0),
                            stop=(k == KT - 1),
                        )
                        nc.tensor.matmul(
                            pq,
                            lhsT=w1_sb[d][:, KT + k, :],
                            rhs=reprT_sb[k],
                            start=(k == 0),
                            stop=(k == KT - 1),
                        )
                    pt = cpool.tile([128, N], BF16, tag=f"pT{d}", name=f"pT{d}")
                    nc.scalar.activation(
                        pt, pp, mybir.ActivationFunctionType.Identity,
                    )
                    qt = cpool.tile([128, N], F32, tag=f"qbT{d}", name=f"qbT{d}")
                    nc.scalar.activation(
                        qt, pq, mybir.ActivationFunctionType.Identity,
                        bias=b1_sb[:, d:d + 1],
                    )
                    pT.append(pt)
                    qbT.append(qt)

            # ---- main loop: 8 mega-tiles of 16 i's ------------------------
            outT_r = outT[:]  # [L, N, N]
            with tc.tile_pool(name="ps2", bufs=2, space="PSUM") as ps2, \
                 tc.tile_pool(name="work", bufs=2) as wpool:
                po_l = [None] * NTILES

                def emit_evict(t):
                    # bf16 staging (total out DMA 3.3MB fits one HWDGE
                    # queue; SWDGE drains too slowly for the tail).
                    ot = wpool.tile([L, TILE_I * N], BF16, tag="ot",
                                    name=f"ot{t}", bufs=4)
                    nc.scalar.copy(ot, po_l[t])
                    po_l[t] = None
                    nc.sync.dma_start(
                        out=outT_r[:, t * TILE_I:(t + 1) * TILE_I, :],
                        in_=ot,
                    )

                for t in range(NTILES):
                    last = (t == NTILES - 1)
                    # h-gen: 48 per-i ops, d-outer so d=0 ops front-load
                    # while GEMM1 finishes d=1,2.
                    h4 = [[None] * DT for _ in range(TILE_G)]
                    for g in range(TILE_G):
                        for d in range(DT):
                            h4[g][d] = wpool.tile(
                                [128, GROUP * N], BF16, tag=f"h4_{g}_{d}",
                                name=f"h4_{t}_{g}_{d}", bufs=3,
                            )
                    # ACT takes one il-slice of every h4 tile (12 of 48 ops,
                    # spread thin): each tile finishes its 3 DVE + 1 ACT
                    # slices together, so no single tile becomes a long pole
                    # for the in-order PE queue. (GPSIMD h-gen was tried and
                    # is catastrophic: it contends with DVE's SBUF port.)
                    for d in range(DT):
                        for g in range(TILE_G):
                            for il in range(GROUP):
                                i = t * TILE_I + g * GROUP + il
                                dst = h4[g][d][:, il * N:(il + 1) * N]
                                if il == GROUP - 1:
                                    nc.scalar.activation(
                                        dst, pT[d],
                                        mybir.ActivationFunctionType.Relu,
                                        bias=qbT[d][:, i:i + 1],
                                    )
                                else:
                                    nc.vector.tensor_scalar(
                                        dst, pT[d], qbT[d][:, i:i + 1], 0.0,
                                        add, maxop,
                                    )

                    if not last:
                        po = ps2.tile([L, TILE_I * N], F32, tag="po",
                                      name=f"po{t}", bufs=2)
                        po_l[t] = po
                        # d-outer: stationary W2[d] held across TILE_G MMs;
                        # consecutive MMs write different 512-col slices.
                        for d in range(DT):
                            for g in range(TILE_G):
                                nc.tensor.matmul(
                                    po[:, g * GROUP * N:(g + 1) * GROUP * N],
                                    lhsT=w2_sb[d],
                                    rhs=h4[g][d],
                                    start=(d == 0),
                                    stop=(d == DT - 1),
                                )
                        if t >= 1:
                            emit_evict(t - 1)
                    else:
                        # Last tile: same d-outer MMs, but evict in four
                        # 512-col slices (pipelined with their DMAs across
                        # two queues) to shorten the final drain.
                        po = ps2.tile([L, TILE_I * N], F32, tag="po",
                                      name=f"po{t}", bufs=2)
                        po_l[t] = po
                        emit_evict(t - 1)
                        for d in range(DT):
                            for g in range(TILE_G):
                                nc.tensor.matmul(
                                    po[:, g * GROUP * N:(g + 1) * GROUP * N],
                                    lhsT=w2_sb[d],
                                    rhs=h4[g][d],
                                    start=(d == 0),
                                    stop=(d == DT - 1),
                                )
                        for g in range(TILE_G):
                            otg = wpool.tile([L, GROUP * N], BF16, tag="otg",
                                             name=f"otg{g}", bufs=4)
                            nc.scalar.copy(
                                otg, po[:, g * GROUP * N:(g + 1) * GROUP * N]
                            )
                            i0 = t * TILE_I + g * GROUP
                            nc.sync.dma_start(
                                out=outT_r[:, i0:i0 + GROUP, :],
                                in_=otg,
                            )
                        po_l[t] = None
    # Bacc defers register allocation + wait legalization to finalize().
    nc.finalize()
    return nc


def kernel(repr_w, W1, b1, W2, b2):
    global LAST_RESULT
    repr_w = np.asarray(repr_w, dtype=np.float32)
    W1 = np.asarray(W1, dtype=np.float32)
    b1 = np.asarray(b1, dtype=np.float32)
    W2 = np.asarray(W2, dtype=np.float32)
    b2 = np.asarray(b2, dtype=np.float32)

    nc = _build_program()

    # w1p[d][p, (half,k)*128+c] = W1[half*H + k*128 + p, d*128 + c]
    w1_r = W1.reshape(2, KT, 128, DT, 128)             # [half,k,p,d,c]
    w1p = np.ascontiguousarray(
        w1_r.transpose(3, 2, 0, 1, 4).reshape(DT, 128, 2 * KT * 128)
    ).astype(ml_dtypes.bfloat16)
    # w2p[p, d*L+l] = W2[d*128+p, l]
    w2p = np.ascontiguousarray(
        W2.reshape(DT, 128, L).transpose(1, 0, 2).reshape(128, DT * L)
    ).astype(ml_dtypes.bfloat16)
    # b1 as per-partition columns: col d = b1[d*128:(d+1)*128]
    b1c = np.ascontiguousarray(b1.reshape(DT, 128).T).astype(np.float32)

    in_maps = []
    for c in range(NCORES):
        # reprP[p, k*N+n] = repr_w[c][n, k*128+p]
        rp = np.ascontiguousarray(
            repr_w[c].T.reshape(KT, 128, N).transpose(1, 0, 2)
            .reshape(128, KT * N)
        ).astype(ml_dtypes.bfloat16)
        in_maps.append({
            "reprP": rp,
            "w1p": w1p,
            "b1c": b1c,
            "w2p": w2p,
        })

    res = run_bass_kernel_spmd(nc, in_maps, core_ids=list(range(NCORES)))
    LAST_RESULT = res

    # outT[l, i, j] bf16 -> out[i, j, l] fp32
    out = np.stack(
        [np.moveaxis(res.results[c]["outT"].astype(np.float32), 0, -1)
         for c in range(NCORES)],
        axis=0,
    )
    if np.any(b2):
        out = out + b2[None, None, None, :]
    return np.ascontiguousarray(out, dtype=np.float32)


if __name__ == "__main__":
    rng = np.random.default_rng(0)
    inputs = {
        "repr_w": rng.standard_normal((B, N, H), dtype=np.float32),
        "W1": (rng.standard_normal((2 * H, HID)) * 0.02).astype(np.float32),
        "b1": np.zeros(HID, np.float32),
        "W2": (rng.standard_normal((HID, L)) * 0.02).astype(np.float32),
        "b2": np.zeros(L, np.float32),
    }
    outv = kernel(**inputs)
    print("out", outv.shape, outv.dtype, float(np.abs(outv).max()))


# revision 24
# speedup vs baseline: 1.1612x; 1.1612x over previous
"""Trainium2 Bass kernel for nn_BERTCharting (pairwise-concat MLP).

Reference computation (per batch b):
    p = repr_w[b] @ W1[:H]        # [N, HID]
    q = repr_w[b] @ W1[H:]        # [N, HID]
    h[i,j,:] = relu(p[j] + q[i] + b1)
    out[i,j,:] = h[i,j] @ W2 + b2

Sharding: data-parallel over batch B=8 across the 8 NeuronCores (one batch
element per core). No collectives.

Per-core pipeline (core = batch b; ~70us HW time, rel err ~2e-3 vs fp32):
  - inputs host-prepped: reprT = repr_w[b].T in bf16, W1/W2 bf16, b1 as
    3 per-partition fp32 columns.
  - first GEMM on PE: pT[d, n] / qT[d, n] accumulated over 6 contraction
    tiles in PSUM (fp32); ScalarE evicts pT to SBUF bf16 and qbT = qT + b1
    to SBUF fp32 (bias fused via ACTIVATE Identity).
  - main loop, groups of 4 i's: h[d-tile][128, 4*128] bf16 built by
    relu(pT + qb_col): VectorE dual-op tensor_scalar (add+max0, 2x mode,
    ~167ns/op) for 3 of 4 i's, ScalarE ACTIVATE Relu+bias for i%4==0
    (engine balance). PE: psum[l=100, (i,j)=512] += W2d.T @ h4 over the
    3 d-tiles (B-style: 100-col stationary, 512-col moving, ~221ns/MM).
    Two groups share a 2-bank psum pair; ScalarE evicts [100, 1024] fp32;
    one 400 KB HWDGE DMA per pair writes outT[i, l, j] (contiguous 512B
    j-rows; host swaps back to [i, j, l]).
  - steady state is VectorE/ScalarE-bound (the 6.3M-element broadcast
    relu(p+q) stream is the roofline; per-partition-scalar ops cap at the
    DVE 2x mode).
  - b2 is added on host after the gather iff nonzero (spec fills zeros).
"""

import os
import sys

for _p in ("/opt/trn_rl_repo",):
    if _p not in sys.path and os.path.isdir(_p):
        sys.path.insert(0, _p)

import numpy as np
import ml_dtypes

import concourse.mybir as mybir
from concourse import bacc, bass
from concourse.tile import TileContext
from concourse.bass_utils import run_bass_kernel_spmd


def _ensure_ntff_hook():
    """Provide antenv.axon_hooks (NTFF profile get/set) if the image lacks it,
    and install the ctypes-based profile hook against libaxon_pjrt.so so that
    run_bass_kernel_spmd(trace=True) can capture hardware profiles."""
    try:
        from antenv.axon_hooks import get_axon_ntff_profile_hook  # noqa: F401
        return
    except ImportError:
        pass
    import contextlib
    import ctypes
    import types

    mod = types.ModuleType("antenv.axon_hooks")
    holder = {"hook": None}
    mod.set_axon_ntff_profile_hook = lambda h: holder.__setitem__("hook", h)
    mod.get_axon_ntff_profile_hook = lambda: holder["hook"]
    sys.modules["antenv.axon_hooks"] = mod
    try:
        import antenv
        antenv.axon_hooks = mod
    except ImportError:
        pass

    so_path = "/opt/axon/libaxon_pjrt.so"
    if not os.path.exists(so_path):
        return
    lib = ctypes.CDLL(so_path)
    if not hasattr(lib, "axon_start_nrt_profile"):
        return
    lib.axon_start_nrt_profile.argtypes = [
        ctypes.POINTER(ctypes.c_int64),
        ctypes.c_size_t,
    ]
    lib.axon_start_nrt_profile.restype = ctypes.c_int64
    lib.axon_stop_nrt_profile.argtypes = [ctypes.c_char_p]
    lib.axon_stop_nrt_profile.restype = ctypes.c_int64

    @contextlib.contextmanager
    def _hook(output_dir, device_ids):
        import jax

        jax.devices()
        if device_ids:
            ids = (ctypes.c_int64 * len(device_ids))(*device_ids)
            rc = lib.axon_start_nrt_profile(ids, len(device_ids))
        else:
            rc = lib.axon_start_nrt_profile(None, 0)
        if rc != 0:
            raise RuntimeError(f"axon_start_nrt_profile rc={rc}")
        try:
            yield
        finally:
            n = lib.axon_stop_nrt_profile(str(output_dir).encode())
            print(f"ntff profile: {n} file(s) written to {output_dir}",
                  file=sys.stderr)

    mod.set_axon_ntff_profile_hook(_hook)


_ensure_ntff_hook()

B, N, H = 8, 128, 768
HID, L = 384, 100
NCORES = 8
KT = H // 128          # 6 contraction tiles for the first GEMM
DT = HID // 128        # 3 d-tiles
GROUP = 4              # i's per PSUM bank in the main loop
NGROUPS = N // GROUP   # 32

F32 = mybir.dt.float32
BF16 = mybir.dt.bfloat16

# Stash of the last run's BassKernelResults (test harness reads exec_time_ns).
LAST_RESULT = None


def _build_program():
    nc = bacc.Bacc(None, target_bir_lowering=False)

    reprT = nc.declare_dram_parameter("reprT", [H, N], BF16, isOutput=False)
    w1 = nc.declare_dram_parameter("w1", [2 * H, HID], BF16, isOutput=False)
    b1c = nc.declare_dram_parameter("b1c", [128, DT], F32, isOutput=False)
    w2 = nc.declare_dram_parameter("w2", [HID, L], BF16, isOutput=False)
    # Output transposed per i: outT[i, l, j] (host swaps back to [i, j, l]).
    # This makes every DMA chunk a contiguous 512B j-row — line-rate HWDGE.
    outT = nc.declare_dram_parameter("outT", [N, L, N], F32, isOutput=True)

    add = mybir.AluOpType.add
    maxop = mybir.AluOpType.max

    with TileContext(nc) as tc:
        with tc.tile_pool(name="const", bufs=1) as cpool:
            # ---- constant loads (coalesced: one DMA per tensor) -----------
            reprT_big = cpool.tile([128, KT, N], BF16, tag="reprTb",
                                   name="reprTb")
            nc.sync.dma_start(
                out=reprT_big,
                in_=reprT[:].rearrange("(k p) n -> p k n", p=128),
            )
            reprT_sb = [reprT_big[:, k, :] for k in range(KT)]
            w1_big = cpool.tile([128, 2 * KT, HID], BF16, tag="w1b", name="w1b")
            w1_r = w1[:].rearrange("(k p) d -> p k d", p=128)
            for q0 in range(0, 2 * KT, 3):
                nc.sync.dma_start(
                    out=w1_big[:, q0:q0 + 3, :], in_=w1_r[:, q0:q0 + 3, :]
                )
            w1_sb = [w1_big[:, k, :] for k in range(2 * KT)]
            w2_big = cpool.tile([128, DT, L], BF16, tag="w2b", name="w2b")
            nc.sync.dma_start(
                out=w2_big,
                in_=w2[:].rearrange("(k p) l -> p k l", p=128),
            )
            w2_sb = [w2_big[:, d, :] for d in range(DT)]
            b1_sb = cpool.tile([128, DT], F32, tag="b1c", name="b1sb")
            nc.sync.dma_start(out=b1_sb, in_=b1c[:, :])

            # ---- first GEMMs: pT, qbT -------------------------------------
            pT, qbT = [], []
            with tc.tile_pool(name="ps1", bufs=1, space="PSUM") as ps1:
                for d in range(DT):
                    pp = ps1.tile([128, N], F32, tag=f"pp{d}", name=f"pp{d}")
                    pq = ps1.tile([128, N], F32, tag=f"pq{d}", name=f"pq{d}")
                    for k in range(KT):
                        nc.tensor.matmul(
                            pp,
                            lhsT=w1_sb[k][:, d * 128:(d + 1) * 128],
                            rhs=reprT_sb[k],
                            start=(k == 0),
                            stop=(k == KT - 1),
                        )
                    for k in range(KT):
                        nc.tensor.matmul(
                            pq,
                            lhsT=w1_sb[KT + k][:, d * 128:(d + 1) * 128],
                            rhs=reprT_sb[k],
                            start=(k == 0),
                            stop=(k == KT - 1),
                        )
                    pt = cpool.tile([128, N], BF16, tag=f"pT{d}", name=f"pT{d}")
                    nc.scalar.activation(
                        pt, pp, mybir.ActivationFunctionType.Identity,
                    )
                    qt = cpool.tile([128, N], F32, tag=f"qbT{d}", name=f"qbT{d}")
                    nc.scalar.activation(
                        qt, pq, mybir.ActivationFunctionType.Identity,
                        bias=b1_sb[:, d:d + 1],
                    )
                    pT.append(pt)
                    qbT.append(qt)

            # ---- main loop ------------------------------------------------
            # B-style GEMM: stationary = W2 d-tile [128, 100]; moving = h for
            # a group of 4 i's packed along the free dim [128, 4*128].
            # psum po[l=100, (i,j)=512] accumulates over the 3 d-tiles.
            # Emission is software-pipelined: group g's eviction is emitted
            # at the top of iteration g+1 so ScalarE's eviction of g doesn't
            # queue behind ScalarE h-ops of g+1 (in-order engine queues).
            # OG groups share one ot staging tile -> 1 output DMA per OG.
            OG = 4            # groups per output staging tile / DMA
            PAIR = 2          # psum groups per 2-bank tile / eviction
            outT_r = outT[:].rearrange("i l j -> l i j")
            with tc.tile_pool(name="ps2", bufs=3, space="PSUM") as ps2, \
                 tc.tile_pool(name="work", bufs=8) as wpool:
                po_l = [None] * (NGROUPS // PAIR)
                ot_l = [None] * (NGROUPS // OG)

                def emit_evict(pr):
                    # evict the 2-group psum pair pr -> ot -> 400 KB DMA
                    gbase = pr * PAIR
                    ot = wpool.tile(
                        [L, PAIR * GROUP, N], F32, tag="ot",
                        name=f"ot{pr}", bufs=4,
                    )
                    nc.scalar.copy(ot, po_l[pr])
                    po_l[pr] = None
                    nc.sync.dma_start(
                        out=outT_r[:, gbase * GROUP:(gbase + PAIR) * GROUP, :],
                        in_=ot,
                    )

                for g in range(NGROUPS):
                    h4 = []
                    for d in range(DT):
                        h4d = wpool.tile(
                            [128, GROUP * N], BF16, tag=f"h4_{d}",
                            name=f"h4_{d}_{g}", bufs=16,
                        )
                        h4.append(h4d)
                    for kk in range(GROUP):
                        i = g * GROUP + kk
                        for d in range(DT):
                            dst = h4[d][:, kk * N:(kk + 1) * N]
                            if i % 4 == 0:
                                # relu(pT + qb_col) on ScalarE; kk=0 so these
                                # issue at the head of the group and don't
                                # delay the group's matmuls.
                                nc.scalar.activation(
                                    dst, pT[d],
                                    mybir.ActivationFunctionType.Relu,
                                    bias=qbT[d][:, i:i + 1],
                                )
                            else:
                                nc.vector.tensor_scalar(
                                    dst, pT[d], qbT[d][:, i:i + 1], 0.0,
                                    add, maxop,
                                )
                    if g % PAIR == 0:
                        po_l[g // PAIR] = ps2.tile(
                            [L, PAIR * GROUP * N], F32, tag="po",
                            name=f"po{g // PAIR}",
                        )
                    po = po_l[g // PAIR]
                    half = (g % PAIR) * GROUP * N
                    for d in range(DT):
                        nc.tensor.matmul(
                            po[:, half:half + GROUP * N],
                            lhsT=w2_sb[d],
                            rhs=h4[d],
                            start=(d == 0),
                            stop=(d == DT - 1),
                        )
                    if g % PAIR == PAIR - 1 and g > PAIR:
                        emit_evict(g // PAIR - 1)
                # final pair: two half-evictions so the last DMA is 200 KB
                pr = NGROUPS // PAIR - 1
                gbase = pr * PAIR
                for hh in range(PAIR):
                    oth = wpool.tile([L, GROUP, N], F32, tag="otf",
                                     name=f"otf{hh}", bufs=2)
                    nc.scalar.copy(
                        oth, po_l[pr][:, hh * GROUP * N:(hh + 1) * GROUP * N]
                    )
                    nc.sync.dma_start(
                        out=outT_r[:, (gbase + hh) * GROUP:(gbase + hh + 1) * GROUP, :],
                        in_=oth,
                    )
                po_l[pr] = None
    # Bacc defers register allocation + wait legalization (the 1-wait-per-
    # instruction split) to finalize(); the pjrt run path doesn't call it.
    nc.finalize()
    return nc


def kernel(repr_w, W1, b1, W2, b2):
    global LAST_RESULT
    repr_w = np.asarray(repr_w, dtype=np.float32)
    W1 = np.asarray(W1, dtype=np.float32)
    b1 = np.asarray(b1, dtype=np.float32)
    W2 = np.asarray(W2, dtype=np.float32)
    b2 = np.asarray(b2, dtype=np.float32)

    nc = _build_program()

    w1_bf = W1.astype(ml_dtypes.bfloat16)
    w2_bf = W2.astype(ml_dtypes.bfloat16)
    # b1 as 3 per-partition columns: col d = b1[d*128:(d+1)*128]
    b1c = np.ascontiguousarray(b1.reshape(DT, 128).T).astype(np.float32)

    in_maps = []
    for c in range(NCORES):
        in_maps.append({
            "reprT": np.ascontiguousarray(repr_w[c].T).astype(ml_dtypes.bfloat16),
            "w1": w1_bf,
            "b1c": b1c,
            "w2": w2_bf,
        })

    res = run_bass_kernel_spmd(nc, in_maps, core_ids=list(range(NCORES)))
    LAST_RESULT = res

    # outT[i, l, j] -> out[i, j, l]
    out = np.stack(
        [np.swapaxes(res.results[c]["outT"], 1, 2) for c in range(NCORES)],
        axis=0,
    )
    if np.any(b2):
        out = out + b2[None, None, None, :]
    return np.ascontiguousarray(out, dtype=np.float32)


if __name__ == "__main__":
    rng = np.random.default_rng(0)
    inputs = {
        "repr_w": rng.standard_normal((B, N, H), dtype=np.float32),
        "W1": (rng.standard_normal((2 * H, HID)) * 0.02).astype(np.float32),
        "b1": np.zeros(HID, np.float32),
        "W2": (rng.standard_normal((HID, L)) * 0.02).astype(np.float32),
        "b2": np.zeros(L, np.float32),
    }
    outv = kernel(**inputs)
    print("out", outv.shape, outv.dtype, float(np.abs(outv).max()))



# revision 25
# speedup vs baseline: 1.1766x; 1.0133x over previous
"""Trainium2 Bass kernel for nn_BERTCharting (pairwise-concat MLP).

Reference computation (per batch b):
    p = repr_w[b] @ W1[:H]        # [N, HID]
    q = repr_w[b] @ W1[H:]        # [N, HID]
    h[i,j,:] = relu(p[j] + q[i] + b1)
    out[i,j,:] = h[i,j] @ W2 + b2

Sharding: data-parallel over batch B=8 across the 8 NeuronCores (one batch
element per core). No collectives.

Design (per core; steady state measured DVE 100% / ACT ~100% busy):
  - inputs host-packed so every DMA descriptor is a contiguous >=512B
    per-partition row at full HWDGE rate; load order W1[d=0] -> reprT
    -> W1[d=1,2] (sync queue) so the first GEMM's d=0 chain (and then
    h-gen) starts ~4us after the fixed ~6.7us NEFF preamble; W2
    prefetches on the gpsimd (SWDGE) queue.
  - first GEMM on PE -> pT[d] bf16 / qbT[d]=qT+b1 fp32 (d-sequential;
    p/q accumulation chains interleaved so the PE pipeline overlaps).
  - h-gen: per-i ops [128 dpart, 128 j]: the per-partition scalar
    q[d,i] caps the free dim at N=128 (measured: FD=512 tensor_scalar
    hits the DVE 4x perf mode, but per-i scalars make it unusable, so
    the stream runs at the per-op floor). DVE dual-op tensor_scalar
    ~163 ns/op pipelined; ACT activation(Relu,bias) ~292 ns/op takes
    one il-slice of every h4 tile (12/48 ops) so no single tile is a
    long pole for the in-order PE queue. (GPSIMD tensor_scalar was
    measured at ~2.1us/op and contends with DVE's SBUF port - unusable.)
  - main GEMM: 8 mega-tiles of 16 i's, PSUM po[100, 2048] (4 banks,
    double-buffered), d-OUTER matmul order (stationary W2[d] held for 4
    MMs into different 512-col slices -> no PSUM output dep between
    neighbours). Last tile evicts in four 512-col slices to shorten
    the pipeline drain.
  - eviction: ACT copy [100, 2048] PSUM->SBUF bf16 (~1.9us/tile; DMA
    and GPSIMD have no PSUM route, so eviction must burn ACT/DVE
    cycles), then one DMA with 100 contiguous 4KiB descriptors into
    outT[l, i, j] bf16 (host upcasts + transposes to [i,j,l]; bf16
    halves output DMA, rel err 0.0021 -> 0.0033, budget 2e-2).
  - Note: the chip DVFS-throttles sustained activity (50%-duty windows
    of 11-28us per run), so exec_time is noisy +-2us run-to-run.
"""

import os
import sys

for _p in ("/opt/trn_rl_repo",):
    if _p not in sys.path and os.path.isdir(_p):
        sys.path.insert(0, _p)

import numpy as np
import ml_dtypes

import concourse.mybir as mybir
from concourse import bacc, bass
from concourse.tile import TileContext
from concourse.bass_utils import run_bass_kernel_spmd


def _ensure_ntff_hook():
    """Provide antenv.axon_hooks (NTFF profile get/set) if the image lacks it,
    and install the ctypes-based profile hook against libaxon_pjrt.so so that
    run_bass_kernel_spmd(trace=True) can capture hardware profiles."""
    try:
        from antenv.axon_hooks import get_axon_ntff_profile_hook  # noqa: F401
        return
    except ImportError:
        pass
    import contextlib
    import ctypes
    import types

    mod = types.ModuleType("antenv.axon_hooks")
    holder = {"hook": None}
    mod.set_axon_ntff_profile_hook = lambda h: holder.__setitem__("hook", h)
    mod.get_axon_ntff_profile_hook = lambda: holder["hook"]
    sys.modules["antenv.axon_hooks"] = mod
    try:
        import antenv
        antenv.axon_hooks = mod
    except ImportError:
        pass

    so_path = "/opt/axon/libaxon_pjrt.so"
    if not os.path.exists(so_path):
        return
    lib = ctypes.CDLL(so_path)
    if not hasattr(lib, "axon_start_nrt_profile"):
        return
    lib.axon_start_nrt_profile.argtypes = [
        ctypes.POINTER(ctypes.c_int64),
        ctypes.c_size_t,
    ]
    lib.axon_start_nrt_profile.restype = ctypes.c_int64
    lib.axon_stop_nrt_profile.argtypes = [ctypes.c_char_p]
    lib.axon_stop_nrt_profile.restype = ctypes.c_int64

    @contextlib.contextmanager
    def _hook(output_dir, device_ids):
        import jax

        jax.devices()
        if device_ids:
            ids = (ctypes.c_int64 * len(device_ids))(*device_ids)
            rc = lib.axon_start_nrt_profile(ids, len(device_ids))
        else:
            rc = lib.axon_start_nrt_profile(None, 0)
        if rc != 0:
            raise RuntimeError(f"axon_start_nrt_profile rc={rc}")
        try:
            yield
        finally:
            n = lib.axon_stop_nrt_profile(str(output_dir).encode())
            print(f"ntff profile: {n} file(s) written to {output_dir}",
                  file=sys.stderr)

    mod.set_axon_ntff_profile_hook(_hook)


_ensure_ntff_hook()

B, N, H = 8, 128, 768
HID, L = 384, 100
NCORES = 8
KT = H // 128          # 6 contraction tiles for the first GEMM
DT = HID // 128        # 3 d-tiles
GROUP = 4              # i's per 512-col psum slice
TILE_G = 4             # groups per psum mega-tile (4 banks)
TILE_I = GROUP * TILE_G        # 16 i's per mega-tile
NTILES = N // TILE_I           # 8 mega-tiles

F32 = mybir.dt.float32
BF16 = mybir.dt.bfloat16

# Of the 48 h-gen ops per mega-tile, this many go to ACT (rest DVE);
# balances DVE (~163ns/op) vs ACT (~292ns/op + ~16us eviction load).
ACT_OPS_PER_TILE = 12

# Stash of the last run's BassKernelResults (test harness reads exec_time_ns).
LAST_RESULT = None


def _build_program():
    nc = bacc.Bacc(None, target_bir_lowering=False)

    # Host-packed layouts: per-partition rows contiguous in DRAM.
    reprP = nc.declare_dram_parameter("reprP", [128, KT * N], BF16,
                                      isOutput=False)
    # w1p[d][p, (half,k)*128+c] = W1[half*H + k*128 + p, d*128 + c]
    w1p = nc.declare_dram_parameter("w1p", [DT, 128, 2 * KT * 128], BF16,
                                    isOutput=False)
    # w2p[p, d*L+l] = W2[d*128+p, l]
    w2p = nc.declare_dram_parameter("w2p", [128, DT * L], BF16,
                                    isOutput=False)
    b1c = nc.declare_dram_parameter("b1c", [128, DT], F32, isOutput=False)
    # Output l-major bf16: outT[l, i, j]; host upcasts + transposes to
    # [i, j, l]. bf16 halves the output DMA (~6.5MB -> 3.3MB per core);
    # the added rounding is ~0.4% of scale, well under the 2e-2 budget.
    outT = nc.declare_dram_parameter("outT", [L, N, N], BF16, isOutput=True)

    add = mybir.AluOpType.add
    maxop = mybir.AluOpType.max

    with TileContext(nc) as tc:
        with tc.tile_pool(name="const", bufs=1) as cpool:
            # ---- input loads: one full-rate DMA per chunk ------------------
            # w1[d=0] first so the first GEMM's d=0 chain starts earliest;
            # b1/w2 issued from the idle gpsimd queue to unclog sync.
            w1_sb = []
            w1_tiles = [
                cpool.tile([128, 2 * KT, 128], BF16, tag=f"w1d{d}",
                           name=f"w1d{d}")
                for d in range(DT)
            ]
            nc.sync.dma_start(
                out=w1_tiles[0],
                in_=w1p[0, :, :].rearrange("p (q c) -> p q c", q=2 * KT),
            )
            reprT_big = cpool.tile([128, KT, N], BF16, tag="reprTb",
                                   name="reprTb")
            nc.sync.dma_start(
                out=reprT_big,
                in_=reprP[:].rearrange("p (k n) -> p k n", k=KT),
            )
            reprT_sb = [reprT_big[:, k, :] for k in range(KT)]
            for d in range(1, DT):
                nc.sync.dma_start(
                    out=w1_tiles[d],
                    in_=w1p[d, :, :].rearrange("p (q c) -> p q c", q=2 * KT),
                )
            w1_sb = w1_tiles
            b1_sb = cpool.tile([128, DT], F32, tag="b1c", name="b1sb")
            nc.gpsimd.dma_start(out=b1_sb, in_=b1c[:, :])
            w2_big = cpool.tile([128, DT, L], BF16, tag="w2b", name="w2b")
            nc.gpsimd.dma_start(
                out=w2_big,
                in_=w2p[:].rearrange("p (d l) -> p d l", d=DT),
            )
            w2_sb = [w2_big[:, d, :] for d in range(DT)]

            # ---- first GEMMs: pT, qbT (d-sequential, p/q interleaved) -----
            pT, qbT = [], []
            with tc.tile_pool(name="ps1", bufs=1, space="PSUM") as ps1:
                for d in range(DT):
                    pp = ps1.tile([128, N], F32, tag="pp", name=f"pp{d}",
                                  bufs=2)
                    pq = ps1.tile([128, N], F32, tag="pq", name=f"pq{d}",
                                  bufs=2)
                    for k in range(KT):
                        nc.tensor.matmul(
                            pp,
                            lhsT=w1_sb[d][:, k, :],
                            rhs=reprT_sb[k],
                            start=(k == 0),
                            stop=(k == KT - 1),
                        )
                        nc.tensor.matmul(
                            pq,
                            lhsT=w1_sb[d][:, KT + k, :],
                            rhs=reprT_sb[k],
                            start=(k == 0),
                            stop=(k == KT - 1),
                        )
                    pt = cpool.tile([128, N], BF16, tag=f"pT{d}", name=f"pT{d}")
                    nc.scalar.activation(
                        pt, pp, mybir.ActivationFunctionType.Identity,
                    )
                    qt = cpool.tile([128, N], F32, tag=f"qbT{d}", name=f"qbT{d}")
                    nc.scalar.activation(
                        qt, pq, mybir.ActivationFunctionType.Identity,
                        bias=b1_sb[:, d:d + 1],
                    )
                    pT.append(pt)
                    qbT.append(qt)

            # ---- main loop: 8 mega-tiles of 16 i's ------------------------
            outT_r = outT[:]  # [L, N, N]
            with tc.tile_pool(name="ps2", bufs=2, space="PSUM") as ps2, \
                 tc.tile_pool(name="work", bufs=2) as wpool:
                po_l = [None] * NTILES

                def emit_evict(t):
                    # bf16 staging (total out DMA 3.3MB fits one HWDGE
                    # queue; SWDGE drains too slowly for the tail).
                    ot = wpool.tile([L, TILE_I * N], BF16, tag="ot",
                                    name=f"ot{t}", bufs=4)
                    nc.scalar.copy(ot, po_l[t])
                    po_l[t] = None
                    nc.sync.dma_start(
                        out=outT_r[:, t * TILE_I:(t + 1) * TILE_I, :],
                        in_=ot,
                    )

                for t in range(NTILES):
                    last = (t == NTILES - 1)
                    # h-gen: 48 per-i ops, d-outer so d=0 ops front-load
                    # while GEMM1 finishes d=1,2.
                    h4 = [[None] * DT for _ in range(TILE_G)]
                    for g in range(TILE_G):
                        for d in range(DT):
                            h4[g][d] = wpool.tile(
                                [128, GROUP * N], BF16, tag=f"h4_{g}_{d}",
                                name=f"h4_{t}_{g}_{d}", bufs=3,
                            )
                    # ACT takes one il-slice of every h4 tile (12 of 48 ops,
                    # spread thin): each tile finishes its 3 DVE + 1 ACT
                    # slices together, so no single tile becomes a long pole
                    # for the in-order PE queue. (GPSIMD h-gen was tried and
                    # is catastrophic: it contends with DVE's SBUF port.)
                    for d in range(DT):
                        for g in range(TILE_G):
                            for il in range(GROUP):
                                i = t * TILE_I + g * GROUP + il
                                dst = h4[g][d][:, il * N:(il + 1) * N]
                                if il == GROUP - 1:
                                    nc.scalar.activation(
                                        dst, pT[d],
                                        mybir.ActivationFunctionType.Relu,
                                        bias=qbT[d][:, i:i + 1],
                                    )
                                else:
                                    nc.vector.tensor_scalar(
                                        dst, pT[d], qbT[d][:, i:i + 1], 0.0,
                                        add, maxop,
                                    )

                    if not last:
                        po = ps2.tile([L, TILE_I * N], F32, tag="po",
                                      name=f"po{t}", bufs=2)
                        po_l[t] = po
                        # d-outer: stationary W2[d] held across TILE_G MMs;
                        # consecutive MMs write different 512-col slices.
                        for d in range(DT):
                            for g in range(TILE_G):
                                nc.tensor.matmul(
                                    po[:, g * GROUP * N:(g + 1) * GROUP * N],
                                    lhsT=w2_sb[d],
                                    rhs=h4[g][d],
                                    start=(d == 0),
                                    stop=(d == DT - 1),
                                )
                        if t >= 1:
                            emit_evict(t - 1)
                    else:
                        # Last tile: same d-outer MMs, but evict in four
                        # 512-col slices (pipelined with their DMAs across
                        # two queues) to shorten the final drain.
                        po = ps2.tile([L, TILE_I * N], F32, tag="po",
                                      name=f"po{t}", bufs=2)
                        po_l[t] = po
                        emit_evict(t - 1)
                        for d in range(DT):
                            for g in range(TILE_G):
                                nc.tensor.matmul(
                                    po[:, g * GROUP * N:(g + 1) * GROUP * N],
                                    lhsT=w2_sb[d],
                                    rhs=h4[g][d],
                                    start=(d == 0),
                                    stop=(d == DT - 1),
                                )
                        for g in range(TILE_G):
                            otg = wpool.tile([L, GROUP * N], BF16, tag="otg",
                                             name=f"otg{g}", bufs=4)
                            nc.scalar.copy(
                                otg, po[:, g * GROUP * N:(g + 1) * GROUP * N]
                            )
                            i0 = t * TILE_I + g * GROUP
                            nc.sync.dma_start(
                                out=outT_r[:, i0:i0 + GROUP, :],
                                in_=otg,
                            )
                        po_l[t] = None
    # Bacc defers register allocation + wait legalization to finalize().
    nc.finalize()
    return nc


def kernel(repr_w, W1, b1, W2, b2):
    global LAST_RESULT
    repr_w = np.asarray(repr_w, dtype=np.float32)
    W1 = np.asarray(W1, dtype=np.float32)
    b1 = np.asarray(b1, dtype=np.float32)
    W2 = np.asarray(W2, dtype=np.float32)
    b2 = np.asarray(b2, dtype=np.float32)

    nc = _build_program()

    # w1p[d][p, (half,k)*128+c] = W1[half*H + k*128 + p, d*128 + c]
    w1_r = W1.reshape(2, KT, 128, DT, 128)             # [half,k,p,d,c]
    w1p = np.ascontiguousarray(
        w1_r.transpose(3, 2, 0, 1, 4).reshape(DT, 128, 2 * KT * 128)
    ).astype(ml_dtypes.bfloat16)
    # w2p[p, d*L+l] = W2[d*128+p, l]
    w2p = np.ascontiguousarray(
        W2.reshape(DT, 128, L).transpose(1, 0, 2).reshape(128, DT * L)
    ).astype(ml_dtypes.bfloat16)
    # b1 as per-partition columns: col d = b1[d*128:(d+1)*128]
    b1c = np.ascontiguousarray(b1.reshape(DT, 128).T).astype(np.float32)

    in_maps = []
    for c in range(NCORES):
        # reprP[p, k*N+n] = repr_w[c][n, k*128+p]
        rp = np.ascontiguousarray(
            repr_w[c].T.reshape(KT, 128, N).transpose(1, 0, 2)
            .reshape(128, KT * N)
        ).astype(ml_dtypes.bfloat16)
        in_maps.append({
            "reprP": rp,
            "w1p": w1p,
            "b1c": b1c,
            "w2p": w2p,
        })

    res = run_bass_kernel_spmd(nc, in_maps, core_ids=list(range(NCORES)))
    LAST_RESULT = res

    # outT[l, i, j] bf16 -> out[i, j, l] fp32
    out = np.stack(
        [np.moveaxis(res.results[c]["outT"].astype(np.float32), 0, -1)
         for c in range(NCORES)],
        axis=0,
    )
    if np.any(b2):
        out = out + b2[None, None, None, :]
    return np.ascontiguousarray(out, dtype=np.float32)


if __name__ == "__main__":
    rng = np.random.default_rng(0)
    inputs = {
        "repr_w": rng.standard_normal((B, N, H), dtype=np.float32),
        "W1": (rng.standard_normal((2 * H, HID)) * 0.02).astype(np.float32),
        "b1": np.zeros(HID, np.float32),
        "W2": (rng.standard_normal((HID, L)) * 0.02).astype(np.float32),
        "b2": np.zeros(L, np.float32),
    }
    outv = kernel(**inputs)
    print("out", outv.shape, outv.dtype, float(np.abs(outv).max()))


# revision 27
# speedup vs baseline: 1.2003x; 1.0202x over previous
"""Trainium2 Bass kernel for nn_BERTCharting (pairwise-concat MLP).

Reference computation (per batch b):
    p = repr_w[b] @ W1[:H]        # [N, HID]
    q = repr_w[b] @ W1[H:]        # [N, HID]
    h[i,j,:] = relu(p[j] + q[i] + b1)
    out[i,j,:] = h[i,j] @ W2 + b2

Sharding: data-parallel over batch B=8 across the 8 NeuronCores (one batch
element per core). No collectives.

Design (per core; steady state measured DVE 100% / ACT ~100% busy):
  - inputs host-packed so every DMA descriptor is a contiguous >=512B
    per-partition row at full HWDGE rate; load order W1[d=0] -> reprT
    -> W1[d=1,2] (sync queue) so the first GEMM's d=0 chain (and then
    h-gen) starts ~4us after the fixed ~6.7us NEFF preamble; W2
    prefetches on the gpsimd (SWDGE) queue.
  - first GEMM on PE -> pT[d] bf16 / qbT[d]=qT+b1 fp32 (d-sequential;
    p/q accumulation chains interleaved so the PE pipeline overlaps).
  - h-gen: per-i ops [128 dpart, 128 j]: the per-partition scalar
    q[d,i] caps the free dim at N=128 (measured: FD=512 tensor_scalar
    hits the DVE 4x perf mode, but per-i scalars make it unusable, so
    the stream runs at the per-op floor). DVE dual-op tensor_scalar
    ~163 ns/op pipelined; ACT activation(Relu,bias) ~292 ns/op takes
    one il-slice of every h4 tile (12/48 ops) so no single tile is a
    long pole for the in-order PE queue. (GPSIMD tensor_scalar was
    measured at ~2.1us/op and contends with DVE's SBUF port - unusable.)
  - main GEMM: 8 mega-tiles of 16 i's, PSUM po[100, 2048] (4 banks,
    double-buffered), d-OUTER matmul order (stationary W2[d] held for 4
    MMs into different 512-col slices -> no PSUM output dep between
    neighbours). Last tile evicts in four 512-col slices to shorten
    the pipeline drain.
  - eviction: ACT copy [100, 2048] PSUM->SBUF bf16 (~1.9us/tile; DMA
    and GPSIMD have no PSUM route, so eviction must burn ACT/DVE
    cycles), then one DMA with 100 contiguous 4KiB descriptors into
    outT[l, i, j] bf16 (host upcasts + transposes to [i,j,l]; bf16
    halves output DMA, rel err 0.0021 -> 0.0033, budget 2e-2).
  - Note: the chip DVFS-throttles sustained activity (50%-duty windows
    of 11-28us per run), so exec_time is noisy +-2us run-to-run.
"""

import os
import sys

for _p in ("/opt/trn_rl_repo",):
    if _p not in sys.path and os.path.isdir(_p):
        sys.path.insert(0, _p)

import numpy as np
import ml_dtypes

import concourse.mybir as mybir
from concourse import bacc, bass
from concourse.tile import TileContext
from concourse.bass_utils import run_bass_kernel_spmd


def _ensure_ntff_hook():
    """Provide antenv.axon_hooks (NTFF profile get/set) if the image lacks it,
    and install the ctypes-based profile hook against libaxon_pjrt.so so that
    run_bass_kernel_spmd(trace=True) can capture hardware profiles."""
    try:
        from antenv.axon_hooks import get_axon_ntff_profile_hook  # noqa: F401
        return
    except ImportError:
        pass
    import contextlib
    import ctypes
    import types

    mod = types.ModuleType("antenv.axon_hooks")
    holder = {"hook": None}
    mod.set_axon_ntff_profile_hook = lambda h: holder.__setitem__("hook", h)
    mod.get_axon_ntff_profile_hook = lambda: holder["hook"]
    sys.modules["antenv.axon_hooks"] = mod
    try:
        import antenv
        antenv.axon_hooks = mod
    except ImportError:
        pass

    so_path = "/opt/axon/libaxon_pjrt.so"
    if not os.path.exists(so_path):
        return
    lib = ctypes.CDLL(so_path)
    if not hasattr(lib, "axon_start_nrt_profile"):
        return
    lib.axon_start_nrt_profile.argtypes = [
        ctypes.POINTER(ctypes.c_int64),
        ctypes.c_size_t,
    ]
    lib.axon_start_nrt_profile.restype = ctypes.c_int64
    lib.axon_stop_nrt_profile.argtypes = [ctypes.c_char_p]
    lib.axon_stop_nrt_profile.restype = ctypes.c_int64

    @contextlib.contextmanager
    def _hook(output_dir, device_ids):
        import jax

        jax.devices()
        if device_ids:
            ids = (ctypes.c_int64 * len(device_ids))(*device_ids)
            rc = lib.axon_start_nrt_profile(ids, len(device_ids))
        else:
            rc = lib.axon_start_nrt_profile(None, 0)
        if rc != 0:
            raise RuntimeError(f"axon_start_nrt_profile rc={rc}")
        try:
            yield
        finally:
            n = lib.axon_stop_nrt_profile(str(output_dir).encode())
            print(f"ntff profile: {n} file(s) written to {output_dir}",
                  file=sys.stderr)

    mod.set_axon_ntff_profile_hook(_hook)


_ensure_ntff_hook()

B, N, H = 8, 128, 768
HID, L = 384, 100
NCORES = 8
KT = H // 128          # 6 contraction tiles for the first GEMM
DT = HID // 128        # 3 d-tiles
GROUP = 4              # i's per 512-col psum slice
TILE_G = 4             # groups per psum mega-tile (4 banks)
TILE_I = GROUP * TILE_G        # 16 i's per mega-tile
NTILES = N // TILE_I           # 8 mega-tiles

F32 = mybir.dt.float32
BF16 = mybir.dt.bfloat16

# Of the 48 h-gen ops per mega-tile, this many go to ACT (rest DVE);
# balances DVE (~163ns/op) vs ACT (~292ns/op + ~16us eviction load).
ACT_OPS_PER_TILE = 12

# Stash of the last run's BassKernelResults (test harness reads exec_time_ns).
LAST_RESULT = None


def _build_program():
    nc = bacc.Bacc(None, target_bir_lowering=False)

    # Host-packed layouts: per-partition rows contiguous in DRAM.
    reprP = nc.declare_dram_parameter("reprP", [128, KT * N], BF16,
                                      isOutput=False)
    # w1p[d][p, (half,k)*128+c] = W1[half*H + k*128 + p, d*128 + c]
    w1p = nc.declare_dram_parameter("w1p", [DT, 128, 2 * KT * 128], BF16,
                                    isOutput=False)
    # w2p[p, d*L+l] = W2[d*128+p, l]
    w2p = nc.declare_dram_parameter("w2p", [128, DT * L], BF16,
                                    isOutput=False)
    b1c = nc.declare_dram_parameter("b1c", [128, DT], F32, isOutput=False)
    # Output l-major bf16: outT[l, i, j]; host upcasts + transposes to
    # [i, j, l]. bf16 halves the output DMA (~6.5MB -> 3.3MB per core);
    # the added rounding is ~0.4% of scale, well under the 2e-2 budget.
    outT = nc.declare_dram_parameter("outT", [L, N, N], BF16, isOutput=True)

    add = mybir.AluOpType.add
    maxop = mybir.AluOpType.max

    with TileContext(nc) as tc:
        with tc.tile_pool(name="const", bufs=1) as cpool:
            # ---- input loads: one full-rate DMA per chunk ------------------
            # w1[d=0] first so the first GEMM's d=0 chain starts earliest;
            # b1/w2 issued from the idle gpsimd queue to unclog sync.
            w1_sb = []
            w1_tiles = [
                cpool.tile([128, 2 * KT, 128], BF16, tag=f"w1d{d}",
                           name=f"w1d{d}")
                for d in range(DT)
            ]
            nc.sync.dma_start(
                out=w1_tiles[0],
                in_=w1p[0, :, :].rearrange("p (q c) -> p q c", q=2 * KT),
            )
            reprT_big = cpool.tile([128, KT, N], BF16, tag="reprTb",
                                   name="reprTb")
            nc.sync.dma_start(
                out=reprT_big,
                in_=reprP[:].rearrange("p (k n) -> p k n", k=KT),
            )
            reprT_sb = [reprT_big[:, k, :] for k in range(KT)]
            for d in range(1, DT):
                nc.sync.dma_start(
                    out=w1_tiles[d],
                    in_=w1p[d, :, :].rearrange("p (q c) -> p q c", q=2 * KT),
                )
            w1_sb = w1_tiles
            b1_sb = cpool.tile([128, DT], F32, tag="b1c", name="b1sb")
            nc.gpsimd.dma_start(out=b1_sb, in_=b1c[:, :])
            w2_big = cpool.tile([128, DT, L], BF16, tag="w2b", name="w2b")
            nc.gpsimd.dma_start(
                out=w2_big,
                in_=w2p[:].rearrange("p (d l) -> p d l", d=DT),
            )
            w2_sb = [w2_big[:, d, :] for d in range(DT)]

            # ---- first GEMMs: pT, qbT (d-sequential, p/q interleaved) -----
            pT, qbT = [], []
            with tc.tile_pool(name="ps1", bufs=1, space="PSUM") as ps1:
                for d in range(DT):
                    pp = ps1.tile([128, N], F32, tag="pp", name=f"pp{d}",
                                  bufs=2)
                    pq = ps1.tile([128, N], F32, tag="pq", name=f"pq{d}",
                                  bufs=2)
                    for k in range(KT):
                        nc.tensor.matmul(
                            pp,
                            lhsT=w1_sb[d][:, k, :],
                            rhs=reprT_sb[k],
                            start=(k == 0),
                            stop=(k == KT - 1),
                        )
                        nc.tensor.matmul(
                            pq,
                            lhsT=w1_sb[d][:, KT + k, :],
                            rhs=reprT_sb[k],
                            start=(k == 0),
                            stop=(k == KT - 1),
                        )
                    pt = cpool.tile([128, N], BF16, tag=f"pT{d}", name=f"pT{d}")
                    qt = cpool.tile([128, N], F32, tag=f"qbT{d}", name=f"qbT{d}")
                    if d == 0:
                        # d=0 evictions on DVE (idle during the ramp): the
                        # first h-gen ops then follow on the same in-order
                        # queue with no cross-engine handoff.
                        nc.vector.tensor_scalar(
                            pt, pp, 0.0, None, add, mybir.AluOpType.bypass,
                        )
                        nc.vector.tensor_scalar(
                            qt, pq, b1_sb[:, d:d + 1], None, add,
                            mybir.AluOpType.bypass,
                        )
                    else:
                        nc.scalar.activation(
                            pt, pp, mybir.ActivationFunctionType.Identity,
                        )
                        nc.scalar.activation(
                            qt, pq, mybir.ActivationFunctionType.Identity,
                            bias=b1_sb[:, d:d + 1],
                        )
                    pT.append(pt)
                    qbT.append(qt)

            # ---- main loop: 8 mega-tiles of 16 i's ------------------------
            outT_r = outT[:]  # [L, N, N]
            with tc.tile_pool(name="ps2", bufs=2, space="PSUM") as ps2, \
                 tc.tile_pool(name="work", bufs=2) as wpool:
                po_l = [None] * NTILES

                def emit_evict(t):
                    # bf16 staging (total out DMA 3.3MB fits one HWDGE
                    # queue; SWDGE drains too slowly for the tail).
                    ot = wpool.tile([L, TILE_I * N], BF16, tag="ot",
                                    name=f"ot{t}", bufs=4)
                    nc.scalar.copy(ot, po_l[t])
                    po_l[t] = None
                    nc.sync.dma_start(
                        out=outT_r[:, t * TILE_I:(t + 1) * TILE_I, :],
                        in_=ot,
                    )

                for t in range(NTILES):
                    last = (t == NTILES - 1)
                    # h-gen: 48 per-i ops, d-outer so d=0 ops front-load
                    # while GEMM1 finishes d=1,2.
                    h4 = [[None] * DT for _ in range(TILE_G)]
                    for g in range(TILE_G):
                        for d in range(DT):
                            h4[g][d] = wpool.tile(
                                [128, GROUP * N], BF16, tag=f"h4_{g}_{d}",
                                name=f"h4_{t}_{g}_{d}", bufs=3,
                            )
                    # ACT takes one il-slice of every h4 tile (12 of 48 ops,
                    # spread thin): each tile finishes its 3 DVE + 1 ACT
                    # slices together, so no single tile becomes a long pole
                    # for the in-order PE queue. (GPSIMD h-gen was tried and
                    # is catastrophic: it contends with DVE's SBUF port.)
                    # Tile 0: ACT has no eviction yet -> give it 18 ops.
                    # Last tile: ACT has the tail evictions -> give it 8.
                    def act_sel(d, g, il):
                        if t == 0:
                            return il == GROUP - 1 or (il == 1 and g < 2)
                        if last:
                            return il == GROUP - 1 and d < 2
                        return il == GROUP - 1
                    for d in range(DT):
                        for g in range(TILE_G):
                            for il in range(GROUP):
                                i = t * TILE_I + g * GROUP + il
                                dst = h4[g][d][:, il * N:(il + 1) * N]
                                if act_sel(d, g, il):
                                    nc.scalar.activation(
                                        dst, pT[d],
                                        mybir.ActivationFunctionType.Relu,
                                        bias=qbT[d][:, i:i + 1],
                                    )
                                else:
                                    nc.vector.tensor_scalar(
                                        dst, pT[d], qbT[d][:, i:i + 1], 0.0,
                                        add, maxop,
                                    )

                    if not last:
                        po = ps2.tile([L, TILE_I * N], F32, tag="po",
                                      name=f"po{t}", bufs=2)
                        po_l[t] = po
                        # d-outer: stationary W2[d] held across TILE_G MMs;
                        # consecutive MMs write different 512-col slices.
                        for d in range(DT):
                            for g in range(TILE_G):
                                nc.tensor.matmul(
                                    po[:, g * GROUP * N:(g + 1) * GROUP * N],
                                    lhsT=w2_sb[d],
                                    rhs=h4[g][d],
                                    start=(d == 0),
                                    stop=(d == DT - 1),
                                )
                        if t >= 1:
                            emit_evict(t - 1)
                    else:
                        # Last tile: same d-outer MMs, but evict in four
                        # 512-col slices (pipelined with their DMAs across
                        # two queues) to shorten the final drain.
                        po = ps2.tile([L, TILE_I * N], F32, tag="po",
                                      name=f"po{t}", bufs=2)
                        po_l[t] = po
                        emit_evict(t - 1)
                        for d in range(DT):
                            for g in range(TILE_G):
                                nc.tensor.matmul(
                                    po[:, g * GROUP * N:(g + 1) * GROUP * N],
                                    lhsT=w2_sb[d],
                                    rhs=h4[g][d],
                                    start=(d == 0),
                                    stop=(d == DT - 1),
                                )
                        for g in range(TILE_G):
                            otg = wpool.tile([L, GROUP * N], BF16, tag="otg",
                                             name=f"otg{g}", bufs=4)
                            nc.scalar.copy(
                                otg, po[:, g * GROUP * N:(g + 1) * GROUP * N]
                            )
                            i0 = t * TILE_I + g * GROUP
                            nc.sync.dma_start(
                                out=outT_r[:, i0:i0 + GROUP, :],
                                in_=otg,
                            )
                        po_l[t] = None
    # Bacc defers register allocation + wait legalization to finalize().
    nc.finalize()
    return nc


def kernel(repr_w, W1, b1, W2, b2):
    global LAST_RESULT
    repr_w = np.asarray(repr_w, dtype=np.float32)
    W1 = np.asarray(W1, dtype=np.float32)
    b1 = np.asarray(b1, dtype=np.float32)
    W2 = np.asarray(W2, dtype=np.float32)
    b2 = np.asarray(b2, dtype=np.float32)

    nc = _build_program()

    # w1p[d][p, (half,k)*128+c] = W1[half*H + k*128 + p, d*128 + c]
    w1_r = W1.reshape(2, KT, 128, DT, 128)             # [half,k,p,d,c]
    w1p = np.ascontiguousarray(
        w1_r.transpose(3, 2, 0, 1, 4).reshape(DT, 128, 2 * KT * 128)
    ).astype(ml_dtypes.bfloat16)
    # w2p[p, d*L+l] = W2[d*128+p, l]
    w2p = np.ascontiguousarray(
        W2.reshape(DT, 128, L).transpose(1, 0, 2).reshape(128, DT * L)
    ).astype(ml_dtypes.bfloat16)
    # b1 as per-partition columns: col d = b1[d*128:(d+1)*128]
    b1c = np.ascontiguousarray(b1.reshape(DT, 128).T).astype(np.float32)

    in_maps = []
    for c in range(NCORES):
        # reprP[p, k*N+n] = repr_w[c][n, k*128+p]
        rp = np.ascontiguousarray(
            repr_w[c].T.reshape(KT, 128, N).transpose(1, 0, 2)
            .reshape(128, KT * N)
        ).astype(ml_dtypes.bfloat16)
        in_maps.append({
            "reprP": rp,
            "w1p": w1p,
            "b1c": b1c,
            "w2p": w2p,
        })

    res = run_bass_kernel_spmd(nc, in_maps, core_ids=list(range(NCORES)))
    LAST_RESULT = res

    # outT[l, i, j] bf16 -> out[i, j, l] fp32
    out = np.stack(
        [np.moveaxis(res.results[c]["outT"].astype(np.float32), 0, -1)
         for c in range(NCORES)],
        axis=0,
    )
    if np.any(b2):
        out = out + b2[None, None, None, :]
    return np.ascontiguousarray(out, dtype=np.float32)


if __name__ == "__main__":
    rng = np.random.default_rng(0)
    inputs = {
        "repr_w": rng.standard_normal((B, N, H), dtype=np.float32),
        "W1": (rng.standard_normal((2 * H, HID)) * 0.02).astype(np.float32),
        "b1": np.zeros(HID, np.float32),
        "W2": (rng.standard_normal((HID, L)) * 0.02).astype(np.float32),
        "b2": np.zeros(L, np.float32),
    }
    outv = kernel(**inputs)
    print("out", outv.shape, outv.dtype, float(np.abs(outv).max()))
